# revision 22
# baseline (speedup 1.0000x reference)
"""BiLSTM-CRF forward loss on 8 Trainium2 NeuronCores.

Sharding: data-parallel on batch. 8 cores x 4 sequences each; each core runs
embedding gather (indirect DMA), both LSTM directions (backward direction via
host-prepared reversed token order), the FC projection, log_softmax, and the
full CRF (emission score sum + forward-algorithm logZ) on device, emitting
just [logZ, emit_sum] per sequence. The host adds the transition/start/end
lookups (pure y/seq_len table lookups, no device data needed).

Dispatch path: a single jitted shard_map callable is built once and reused
across calls; device-resident inputs are cached keyed by a content hash of
the source arrays, so steady-state calls ship no bulk data over the axon
tunnel (the 32MB embedding table ships once, on the first call).
"""

import os
os.environ.setdefault("BASS_NEVER_TRACE", "1")
import zlib
import numpy as np
import time as _time
from contextlib import ExitStack
from concurrent.futures import ThreadPoolExecutor

import jax
import concourse.bass as bass
import concourse.bacc as bacc
import concourse.mybir as mybir
from concourse import tile
from concourse.bass2jax import (_bass_exec_p, install_neuronx_cc_hook,
                                partition_id_tensor, Mesh, PartitionSpec,
                                shard_map)
from jax.sharding import NamedSharding

B, T, V, E, H, C = 32, 512, 32000, 256, 256, 20
NCORES = 8
BL = B // NCORES          # 4 sequences per core
NTOK = BL * T             # 2048 tokens per core
NTILE = NTOK // 128       # 16 token tiles
F32 = mybir.dt.float32
BF16 = mybir.dt.bfloat16
I32 = mybir.dt.int32
NPBF16 = mybir.dt.np(mybir.dt.bfloat16)
AF = mybir.ActivationFunctionType
ALU = mybir.AluOpType
AX = mybir.AxisListType

# gate permutation: torch order i,f,g,o -> i,f,o,g (sigmoid block contiguous)
GPERM = np.concatenate([np.arange(0, 256), np.arange(256, 512),
                        np.arange(768, 1024), np.arange(512, 768)])

NORM_EVERY = 8            # renormalize the exp-domain alpha every k steps

_cache = {}


def _build_nc():
    nc = bacc.Bacc()
    emb_d = nc.declare_dram_parameter("emb", [V, E], F32, isOutput=False)
    idx_d = {d: nc.declare_dram_parameter(f"idx{d}", [NTILE, 128, 1], I32,
                                          isOutput=False) for d in (0, 1)}
    wih_d = {d: nc.declare_dram_parameter(f"wih{d}", [128, 2048], BF16,
                                          isOutput=False) for d in (0, 1)}
    whh_d = {d: nc.declare_dram_parameter(f"whh{d}", [128, 2048], BF16,
                                          isOutput=False) for d in (0, 1)}
    bias_d = {d: nc.declare_dram_parameter(f"bias{d}", [128, 8], F32,
                                           isOutput=False) for d in (0, 1)}
    wfc_d = {d: nc.declare_dram_parameter(f"wfc{d}", [128, 40], BF16,
                                          isOutput=False) for d in (0, 1)}
    ident_d = nc.declare_dram_parameter("ident", [128, 128], F32, isOutput=False)
    idxg_d = nc.declare_dram_parameter("idxg", [NTILE, 128, 1], I32,
                                       isOutput=False)
    oh_d = nc.declare_dram_parameter("oh", [NTILE, 128, C], F32, isOutput=False)
    maskb_d = nc.declare_dram_parameter("maskb", [C, NTOK], I32, isOutput=False)
    expT_d = nc.declare_dram_parameter("expT", [128, C], F32, isOutput=False)
    onesp_d = nc.declare_dram_parameter("onesp", [128, C], F32, isOutput=False)
    expS_d = nc.declare_dram_parameter("expS", [C, BL], F32, isOutput=False)
    expE_d = nc.declare_dram_parameter("expE", [C, BL], F32, isOutput=False)
    bsel_d = nc.declare_dram_parameter("bsel", [128, BL], F32, isOutput=False)
    out_d = nc.declare_dram_parameter("out", [2, BL], F32, isOutput=True)

    with ExitStack() as ctx:
        tc = ctx.enter_context(tile.TileContext(nc))
        const_p = ctx.enter_context(tc.tile_pool(name="const", bufs=1))
        xp_p = ctx.enter_context(tc.tile_pool(name="xp", bufs=1))
        hist_p = ctx.enter_context(tc.tile_pool(name="hist", bufs=1))

        ident = const_p.tile([128, 128], F32, tag="ident")
        nc.sync.dma_start(out=ident[:], in_=ident_d[:])
        wih, whh, bias, wfc, xp, hist, cst = {}, {}, {}, {}, {}, {}, {}
        for d in (0, 1):
            wih[d] = const_p.tile([128, 2048], BF16, tag=f"wih{d}", name=f"wih_sb{d}")
            whh[d] = const_p.tile([128, 2048], BF16, tag=f"whh{d}", name=f"whh_sb{d}")
            bias[d] = const_p.tile([128, 8], F32, tag=f"bias{d}", name=f"bias_sb{d}")
            wfc[d] = const_p.tile([128, 40], BF16, tag=f"wfc{d}", name=f"wfc_sb{d}")
            nc.sync.dma_start(out=wih[d][:], in_=wih_d[d][:])
            nc.sync.dma_start(out=whh[d][:], in_=whh_d[d][:])
            nc.sync.dma_start(out=bias[d][:], in_=bias_d[d][:])
            nc.sync.dma_start(out=wfc[d][:], in_=wfc_d[d][:])
            # xp[d]: [128, T*32] bf16, col = t*32 + c*4 + b
            xp[d] = xp_p.tile([128, T * 32], BF16, tag=f"xp{d}", name=f"xp_sb{d}")
            # hist[d]: [128, (T+1)*8] bf16, col = t*8 + k*4 + b (slot 0 = h=0)
            hist[d] = hist_p.tile([128, (T + 1) * 8], BF16, tag=f"hist{d}", name=f"hist_sb{d}")
            cst[d] = const_p.tile([128, 8], F32, tag=f"cst{d}", name=f"cst_sb{d}")
            nc.gpsimd.memset(hist[d][:, 0:8], 0.0)
            nc.gpsimd.memset(cst[d][:], 0.0)

        # CRF constants / state
        expT = const_p.tile([128, C], F32, tag="expT")
        onesp = const_p.tile([128, C], F32, tag="onesp")
        expS = const_p.tile([C, BL], F32, tag="expS")
        expE = const_p.tile([C, BL], F32, tag="expE")
        bsel = const_p.tile([128, BL], F32, tag="bsel")
        maskb = const_p.tile([C, NTOK], I32, tag="maskb")
        oh_sb = const_p.tile([128, NTILE * C], F32, tag="ohsb")
        expL = const_p.tile([C, NTOK], F32, tag="expL")
        fwdT = const_p.tile([128, NTILE * C], F32, tag="fwdT")
        emitc = const_p.tile([128, NTILE], F32, tag="emitc")
        alphA = const_p.tile([128, BL], F32, tag="alphA")
        alphB = const_p.tile([128, BL], F32, tag="alphB")
        uFull = const_p.tile([128, BL], F32, tag="uFull")
        nc.sync.dma_start(out=expT[:], in_=expT_d[:])
        nc.sync.dma_start(out=onesp[:], in_=onesp_d[:])
        nc.sync.dma_start(out=expS[:], in_=expS_d[:])
        nc.sync.dma_start(out=expE[:], in_=expE_d[:])
        nc.sync.dma_start(out=bsel[:], in_=bsel_d[:])
        nc.sync.dma_start(out=maskb[:], in_=maskb_d[:])
        for j in range(NTILE):
            nc.sync.dma_start(out=oh_sb[:, j * C:(j + 1) * C], in_=oh_d[j])
        nc.gpsimd.memset(alphA[:], 0.0)
        nc.gpsimd.memset(alphB[:], 0.0)
        nc.gpsimd.memset(uFull[:], 0.0)

        # ---- phase 1+2: gather + transpose + input projection, per dir ----
        for d in (0, 1):
            with tc.tile_pool(name="xeT", bufs=2) as xeT_p, \
                 tc.tile_pool(name="gat", bufs=3) as gat_p, \
                 tc.tile_pool(name="tps", bufs=2, space="PSUM") as tps_p, \
                 tc.tile_pool(name="pps", bufs=2, space="PSUM") as pps_p:
                xeT = [xeT_p.tile([128, NTOK], BF16, tag=f"xeT{k}", name=f"xeT_sb{d}_{k}")
                       for k in (0, 1)]
                for j in range(NTILE):
                    idx_sb = gat_p.tile([128, 1], I32, tag="idx")
                    nc.sync.dma_start(out=idx_sb[:], in_=idx_d[d][j])
                    xe_sb = gat_p.tile([128, E], F32, tag="xe")
                    nc.gpsimd.indirect_dma_start(
                        out=xe_sb[:], out_offset=None, in_=emb_d[:],
                        in_offset=bass.IndirectOffsetOnAxis(ap=idx_sb[:, :1],
                                                            axis=0))
                    for k in (0, 1):
                        ps = tps_p.tile([128, 128], F32, tag="tps")
                        nc.tensor.transpose(ps[:], xe_sb[:, k * 128:(k + 1) * 128],
                                            ident[:])
                        nc.vector.tensor_copy(
                            out=xeT[k][:, j * 128:(j + 1) * 128], in_=ps[:])
                # projection: xpT[g, tok] = Wih_perm @ xe.T + b
                xp3 = xp[d][:].rearrange("p (t x) -> p t x", x=32)
                for cchunk in range(8):
                    for n in range(4):
                        ps = pps_p.tile([128, 512], F32, tag="pps")
                        for k in (0, 1):
                            nc.tensor.matmul(
                                out=ps[:],
                                lhsT=wih[d][:, k * 1024 + cchunk * 128:
                                            k * 1024 + (cchunk + 1) * 128],
                                rhs=xeT[k][:, n * 512:(n + 1) * 512],
                                start=(k == 0), stop=(k == 1))
                        dst = xp3[:, n * 128:(n + 1) * 128,
                                  cchunk * 4:(cchunk + 1) * 4]
                        src = ps[:].rearrange("p (t b) -> p t b", b=4)
                        nc.scalar.activation(
                            dst, src, AF.Identity,
                            bias=bias[d][:, cchunk:cchunk + 1], scale=1.0)

        # ---- phase 3: the two LSTM scans ----
        with tc.tile_pool(name="scan", bufs=3) as scan_p, \
             tc.tile_pool(name="gps", bufs=2, space="PSUM") as gps_p:

            def step(i):
                for d in (0, 1):
                    hcur = scan_p.tile([128, 8], BF16, tag=f"hc{d}", name=f"hcur{d}")
                    nc.vector.tensor_copy(out=hcur[:],
                                          in_=hist[d][:, i * 8:i * 8 + 8])
                    ps = gps_p.tile([128, 32], F32, tag=f"g{d}")
                    for cchunk in range(8):
                        for k in (0, 1):
                            nc.tensor.matmul(
                                out=ps[:, cchunk * 4:(cchunk + 1) * 4],
                                lhsT=whh[d][:, k * 1024 + cchunk * 128:
                                            k * 1024 + (cchunk + 1) * 128],
                                rhs=hcur[:, k * 4:(k + 1) * 4],
                                start=(k == 0), stop=(k == 1))
                    g = scan_p.tile([128, 32], F32, tag=f"gt{d}")
                    nc.vector.tensor_add(out=g[:], in0=ps[:],
                                         in1=xp[d][:, i * 32:(i + 1) * 32])
                    s = scan_p.tile([128, 32], F32, tag=f"sg{d}")
                    nc.scalar.activation(s[:, 0:24], g[:, 0:24], AF.Sigmoid)
                    nc.scalar.activation(s[:, 24:32], g[:, 24:32], AF.Tanh)
                    t1 = scan_p.tile([128, 8], F32, tag=f"t1{d}")
                    t2 = scan_p.tile([128, 8], F32, tag=f"t2{d}")
                    nc.vector.tensor_mul(out=t1[:], in0=s[:, 0:8],
                                         in1=s[:, 24:32])          # i*g~
                    nc.vector.tensor_mul(out=t2[:], in0=s[:, 8:16],
                                         in1=cst[d][:])            # f*c
                    nc.vector.tensor_add(out=cst[d][:], in0=t1[:], in1=t2[:])
                    th = scan_p.tile([128, 8], F32, tag=f"th{d}")
                    nc.scalar.activation(th[:], cst[d][:], AF.Tanh)
                    h = scan_p.tile([128, 8], F32, tag=f"h{d}")
                    nc.vector.tensor_mul(out=h[:], in0=s[:, 16:24], in1=th[:])
                    nc.vector.tensor_copy(
                        out=hist[d][:, i * 8 + 8:i * 8 + 16], in_=h[:])

            for _i in range(T):
                step(_i)

        # ---- phase 4: FC halves; fwd kept transposed in SBUF, bwd scattered
        # to DRAM scratch (then gathered back in natural token order) ----
        with tc.tile_pool(name="fps", bufs=2, space="PSUM") as fps_p, \
             tc.tile_pool(name="fpssb", bufs=2) as fps_sb, \
             tc.tile_pool(name="ftp", bufs=3, space="PSUM") as ftp_p, \
             tc.tile_pool(name="btile", bufs=3) as bt_p, \
             tc.tile_pool(name="scrp", bufs=1, space="DRAM") as scr_p:
            scr = scr_p.tile([NTOK, C], F32, tag="scr")
            for d in (0, 1):
                h3 = hist[d][:].rearrange("p (t x) -> p t x", x=8)
                for n in range(4):
                    ps = fps_p.tile([C, 512], F32, tag="fc")
                    for k in (0, 1):
                        rhs = h3[:, n * 128 + 1:(n + 1) * 128 + 1,
                                 k * 4:k * 4 + 4]
                        nc.tensor.matmul(
                            out=ps[:], lhsT=wfc[d][:, k * 20:(k + 1) * 20],
                            rhs=rhs, start=(k == 0), stop=(k == 1))
                    ob = fps_sb.tile([C, 512], F32, tag="fcsb", name="fc_sb")
                    nc.vector.tensor_copy(out=ob[:], in_=ps[:])
                    for k4 in range(4):
                        j = n * 4 + k4
                        tp = ftp_p.tile([128, C], F32, tag="ftp")
                        nc.tensor.transpose(tp[:], ob[:, k4 * 128:(k4 + 1) * 128],
                                            ident[:C, :C])
                        if d == 0:
                            nc.vector.tensor_copy(
                                out=fwdT[:, j * C:(j + 1) * C], in_=tp[:])
                        else:
                            bt = bt_p.tile([128, C], F32, tag="bt")
                            nc.vector.tensor_copy(out=bt[:], in_=tp[:])
                            nc.sync.dma_start(
                                out=scr[j * 128:(j + 1) * 128, :], in_=bt[:])

            # ---- phase 5: un-reverse gather + log_softmax + emit + expL ----
            with tc.tile_pool(name="lsf", bufs=3) as ls_p, \
                 tc.tile_pool(name="lps", bufs=3, space="PSUM") as lps_p:
                for j in range(NTILE):
                    gi = ls_p.tile([128, 1], I32, tag="gi")
                    nc.sync.dma_start(out=gi[:], in_=idxg_d[j])
                    bg = ls_p.tile([128, C], F32, tag="bg")
                    nc.gpsimd.indirect_dma_start(
                        out=bg[:], out_offset=None, in_=scr[:],
                        in_offset=bass.IndirectOffsetOnAxis(ap=gi[:, :1],
                                                            axis=0))
                    pre = ls_p.tile([128, C], F32, tag="pre")
                    nc.vector.tensor_add(out=pre[:],
                                         in0=fwdT[:, j * C:(j + 1) * C],
                                         in1=bg[:])
                    negmax = ls_p.tile([128, 1], F32, tag="negmax")
                    nc.vector.tensor_reduce(out=negmax[:], in_=pre[:],
                                            axis=AX.X, op=ALU.max, negate=True)
                    ex = ls_p.tile([128, C], F32, tag="ex")
                    ssum = ls_p.tile([128, 1], F32, tag="ssum")
                    nc.scalar.activation(ex[:], pre[:], AF.Exp,
                                         bias=negmax[:, 0:1], scale=1.0,
                                         accum_out=ssum[:, 0:1])
                    rcp = ls_p.tile([128, 1], F32, tag="rcp")
                    nc.vector.reciprocal(out=rcp[:], in_=ssum[:])
                    lg = ls_p.tile([128, 1], F32, tag="lg")
                    nc.scalar.activation(lg[:], ssum[:], AF.Ln)
                    nb = ls_p.tile([128, 1], F32, tag="nb")
                    nc.vector.tensor_sub(out=nb[:], in0=negmax[:], in1=lg[:])
                    logit = ls_p.tile([128, C], F32, tag="logit")
                    nc.scalar.activation(logit[:], pre[:], AF.Identity,
                                         bias=nb[:, 0:1], scale=1.0)
                    prod = ls_p.tile([128, C], F32, tag="prod")
                    nc.vector.tensor_mul(out=prod[:], in0=logit[:],
                                         in1=oh_sb[:, j * C:(j + 1) * C])
                    nc.vector.tensor_reduce(out=emitc[:, j:j + 1], in_=prod[:],
                                            axis=AX.X, op=ALU.add)
                    # expL tile = softmax(pre) = ex * (1/ssum), then transpose
                    expl = ls_p.tile([128, C], F32, tag="expl")
                    nc.scalar.activation(expl[:], ex[:], AF.Copy, bias=0.0,
                                         scale=rcp[:, 0:1])
                    tps2 = lps_p.tile([C, 128], F32, tag="tps2")
                    nc.tensor.transpose(tps2[:], expl[:], ident[:])
                    nc.vector.tensor_copy(out=expL[:, j * 128:(j + 1) * 128],
                                          in_=tps2[:])

        # ---- phase 6: emission score sum -> out[1] ----
        with tc.tile_pool(name="fin", bufs=2) as fin_p, \
             tc.tile_pool(name="finps", bufs=2, space="PSUM") as finps_p:
            eacc = fin_p.tile([128, 1], F32, tag="eacc")
            nc.vector.tensor_reduce(out=eacc[:], in_=emitc[:], axis=AX.X,
                                    op=ALU.add)
            eps = finps_p.tile([BL, 1], F32, tag="eps")
            nc.tensor.matmul(out=eps[:], lhsT=bsel[:], rhs=eacc[:],
                             start=True, stop=True)
            esb = fin_p.tile([BL, 1], F32, tag="esb")
            nc.vector.tensor_copy(out=esb[:], in_=eps[:])
            nc.sync.dma_start(out=out_d[1:2, :], in_=esb[:])

            # ---- phase 7: CRF forward recursion (exp domain) ----
            with tc.tile_pool(name="rec", bufs=3) as rec_p, \
                 tc.tile_pool(name="rps", bufs=3, space="PSUM") as rps_p:
                # init: alphaE_0 = expL[:, 0:BL] * exp(start)
                nc.vector.tensor_mul(out=alphA[0:C, :], in0=expL[:, 0:BL],
                                     in1=expS[:])
                m = rec_p.tile([1, BL], F32, tag="m")
                nc.gpsimd.memset(m[:], 0.0)
                alph = [alphA, alphB]
                cur = 0
                for t in range(1, T):
                    psN = rps_p.tile([C, BL], F32, tag="psN")
                    nc.tensor.matmul(out=psN[:], lhsT=expT[:],
                                     rhs=alph[cur][:], start=True, stop=True)
                    nxt = alph[1 - cur]
                    nc.vector.tensor_copy(out=nxt[0:C, :], in_=alph[cur][0:C, :])
                    uE = rec_p.tile([C, BL], F32, tag="uE")
                    nc.vector.tensor_mul(out=uE[:], in0=psN[:],
                                         in1=expL[:, t * BL:(t + 1) * BL])
                    nc.vector.copy_predicated(
                        out=nxt[0:C, :], mask=maskb[:, t * BL:(t + 1) * BL],
                        data=uE[:])
                    cur = 1 - cur
                    if t % NORM_EVERY == 0:
                        sbc = rps_p.tile([C, BL], F32, tag="sbc")
                        nc.tensor.matmul(out=sbc[:], lhsT=onesp[:],
                                         rhs=alph[cur][:], start=True,
                                         stop=True)
                        rcp2 = rec_p.tile([C, BL], F32, tag="rcp2")
                        nc.vector.reciprocal(out=rcp2[:], in_=sbc[:])
                        nxt = alph[1 - cur]
                        nc.vector.tensor_mul(out=nxt[0:C, :],
                                             in0=alph[cur][0:C, :],
                                             in1=rcp2[:])
                        cur = 1 - cur
                        lg2 = rec_p.tile([1, BL], F32, tag="lg2")
                        nc.scalar.activation(lg2[:], sbc[0:1, :], AF.Ln)
                        m2 = rec_p.tile([1, BL], F32, tag="m")
                        nc.vector.tensor_add(out=m2[:], in0=m[:], in1=lg2[:])
                        m = m2
                # final: logZ = m + log(sum_j alphaE * exp(end))
                nc.vector.tensor_mul(out=uFull[0:C, :], in0=alph[cur][0:C, :],
                                     in1=expE[:])
                sF = rps_p.tile([C, BL], F32, tag="sbc")
                nc.tensor.matmul(out=sF[:], lhsT=onesp[:], rhs=uFull[:],
                                 start=True, stop=True)
                lgF = rec_p.tile([1, BL], F32, tag="lg2")
                nc.scalar.activation(lgF[:], sF[0:1, :], AF.Ln)
                logZ = rec_p.tile([1, BL], F32, tag="logZ")
                nc.vector.tensor_add(out=logZ[:], in0=m[:], in1=lgF[:])
                nc.sync.dma_start(out=out_d[0:1, :], in_=logZ[:])
    nc.finalize()
    return nc


def _prep_w(w):
    # w: [1024, din] fp32 (gate-permuted rows) -> [128, 2048] bf16 lhsT layout
    wp = w[GPERM].astype(np.float32)
    din = wp.shape[1]
    w4 = wp.reshape(8, 128, din // 128, 128)          # [c, m, k, p]
    return np.ascontiguousarray(
        w4.transpose(3, 2, 0, 1).reshape(128, 2048)).astype(NPBF16)


def _hash(a):
    """Content fingerprint: shape/dtype + full crc32 (~9ms for 32MB)."""
    a = np.ascontiguousarray(a)
    flat = a.reshape(-1).view(np.uint8)
    return (a.shape, str(a.dtype), zlib.crc32(flat))


def _builders(x, seq_len, y, mask, emb, Wih_f, Whh_f, b_f, Wih_b, Whh_b, b_b,
              W_fc, start_t, end_t, trans):
    """name -> (content_key, builder fn returning NCORES per-core arrays)."""
    t_idx = np.arange(T)
    rev = np.where(t_idx[None, :] < seq_len[:, None],
                   seq_len[:, None] - 1 - t_idx[None, :], t_idx[None, :])
    kx = _hash(x); ksl = _hash(seq_len); ky = _hash(y); km = _hash(mask)
    out = {}

    out["emb"] = (_hash(emb), lambda: [np.asarray(emb, np.float32)] * NCORES)
    out["ident"] = (("ident",), lambda: [np.eye(128, dtype=np.float32)] * NCORES)

    def idx_builder(d):
        def build():
            res = []
            for core in range(NCORES):
                sl = slice(core * BL, (core + 1) * BL)
                xc = np.asarray(x)[sl].astype(np.int64)
                if d == 1:
                    xc = np.take_along_axis(xc, rev[sl].astype(np.int64),
                                            axis=1)
                res.append(np.ascontiguousarray(xc.T).reshape(
                    NTILE, 128, 1).astype(np.int32))
            return res
        return build

    out["idx0"] = (("i0",) + kx, idx_builder(0))
    out["idx1"] = (("i1",) + kx + ksl, idx_builder(1))

    for d, (Wih, Whh, bv) in enumerate(((Wih_f, Whh_f, b_f),
                                        (Wih_b, Whh_b, b_b))):
        out[f"wih{d}"] = (_hash(Wih),
                          lambda Wih=Wih: [_prep_w(np.asarray(Wih))] * NCORES)
        out[f"whh{d}"] = (_hash(Whh),
                          lambda Whh=Whh: [_prep_w(np.asarray(Whh))] * NCORES)

        def bias_build(bv=bv):
            bp = np.asarray(bv)[GPERM].astype(np.float32)
            return [np.ascontiguousarray(bp.reshape(8, 128).T)] * NCORES
        out[f"bias{d}"] = (_hash(bv), bias_build)

        def wfc_build(d=d):
            half = np.asarray(W_fc, np.float32)[:, d * 256:(d + 1) * 256]
            w4 = half.reshape(C, 2, 128).transpose(2, 1, 0)
            return [np.ascontiguousarray(
                w4.reshape(128, 2 * C)).astype(NPBF16)] * NCORES
        out[f"wfc{d}"] = ((d,) + _hash(W_fc), wfc_build)

    def idxg_build():
        res = []
        p = np.arange(NTOK)
        tv, bv = p // BL, p % BL
        for core in range(NCORES):
            revc = rev[core * BL:(core + 1) * BL]
            gi = (revc[bv, tv] * BL + bv).astype(np.int32)
            res.append(gi.reshape(NTILE, 128, 1))
        return res
    out["idxg"] = (("ig",) + ksl, idxg_build)

    def oh_build():
        res = []
        yv = np.asarray(y).astype(np.int64)
        mf = np.asarray(mask).astype(np.float32)
        for core in range(NCORES):
            sl = slice(core * BL, (core + 1) * BL)
            o = np.zeros((BL, T, C), np.float32)
            np.put_along_axis(o, yv[sl][:, :, None], 1.0, axis=2)
            o *= mf[sl][:, :, None]
            # token order: tok = t*BL + b
            res.append(np.ascontiguousarray(o.transpose(1, 0, 2)).reshape(
                NTILE, 128, C))
        return res
    out["oh"] = (("oh",) + ky + km, oh_build)

    def maskb_build():
        res = []
        mi = np.asarray(mask).astype(np.int32)
        for core in range(NCORES):
            sl = slice(core * BL, (core + 1) * BL)
            row = np.ascontiguousarray(mi[sl].T).reshape(1, NTOK)
            res.append(np.repeat(row, C, axis=0))
        return res
    out["maskb"] = (("mb",) + km, maskb_build)

    def expT_build():
        z = np.zeros((128, C), np.float32)
        z[:C] = np.exp(np.asarray(trans, np.float32))
        return [z] * NCORES
    out["expT"] = (("et",) + _hash(trans), expT_build)

    def onesp_build():
        z = np.zeros((128, C), np.float32)
        z[:C] = 1.0
        return [z] * NCORES
    out["onesp"] = (("op",), onesp_build)

    out["expS"] = (("es",) + _hash(start_t), lambda: [np.repeat(
        np.exp(np.asarray(start_t, np.float32))[:, None], BL, 1)] * NCORES)
    out["expE"] = (("ee",) + _hash(end_t), lambda: [np.repeat(
        np.exp(np.asarray(end_t, np.float32))[:, None], BL, 1)] * NCORES)

    def bsel_build():
        z = np.zeros((128, BL), np.float32)
        z[np.arange(128), np.arange(128) % BL] = 1.0
        return [z] * NCORES
    out["bsel"] = (("bs",), bsel_build)
    return out


class _Runner:
    """Builds the jitted shard_map dispatch once; caches device-resident
    inputs keyed by a content hash of the source arrays."""

    def __init__(self, nc):
        install_neuronx_cc_hook()
        self.nc = nc
        pname = nc.partition_id_tensor.name if nc.partition_id_tensor else None
        in_names, out_names, out_avals = [], [], []
        for alloc in nc.m.functions[0].allocations:
            if not isinstance(alloc, mybir.MemoryLocationSet):
                continue
            name = alloc.memorylocations[0].name
            if alloc.kind == "ExternalInput":
                if name != pname:
                    in_names.append(name)
            elif alloc.kind == "ExternalOutput":
                out_names.append(name)
                out_avals.append(jax.core.ShapedArray(
                    tuple(alloc.tensor_shape), mybir.dt.np(alloc.dtype)))
        self.in_names, self.out_names, self.out_avals = in_names, out_names, out_avals
        all_in = in_names + out_names + ([pname] if pname else [])
        navals = tuple(out_avals)

        def _body(*args):
            operands = list(args)
            if pname is not None:
                operands.append(partition_id_tensor())
            return tuple(_bass_exec_p.bind(
                *operands, out_avals=navals, in_names=tuple(all_in),
                out_names=tuple(out_names), lowering_input_output_aliases=(),
                sim_require_finite=True, sim_require_nnan=True, nc=nc))

        devices = jax.devices()[:NCORES]
        self.mesh = Mesh(np.asarray(devices), ("core",))
        self.sharding = NamedSharding(self.mesh, PartitionSpec("core"))
        nin = len(in_names) + len(out_names)
        self.fn = jax.jit(
            shard_map(_body, mesh=self.mesh,
                      in_specs=(PartitionSpec("core"),) * nin,
                      out_specs=(PartitionSpec("core"),) * len(out_names)),
            keep_unused=True)
        # device-resident zero "output seed" buffers, reused every call
        # (the kernel writes every element of every output)
        self.zeros = [
            jax.device_put(
                np.zeros((NCORES * av.shape[0], *av.shape[1:]), av.dtype),
                self.sharding)
            for av in out_avals]
        self.dev = {}      # name -> device array
        self.keys = {}     # name -> content key
        self.pool = ThreadPoolExecutor(NCORES * 2)
        self.args = None   # prebuilt arg list; invalidated by put()

    def put(self, name, key, builder):
        if self.keys.get(name) != key:
            arrs = builder()
            glob = np.concatenate(arrs, axis=0)
            self.dev[name] = jax.device_put(glob, self.sharding)
            self.keys[name] = key
            self.args = None

    def run(self):
        t0 = _time.perf_counter()
        if self.args is None:
            self.args = [self.dev[n] for n in self.in_names] + self.zeros
        out = self.fn(*self.args)
        t1 = _time.perf_counter()
        # fetch all shards of all outputs concurrently (overlaps the
        # per-shard axon round-trip latency)
        work = [(i, s) for i, o in enumerate(out)
                for s in o.addressable_shards]
        datas = list(self.pool.map(lambda w: np.asarray(w[1].data), work))
        res = [np.empty((NCORES * av.shape[0], *av.shape[1:]), av.dtype)
               for av in self.out_avals]
        for (i, s), d in zip(work, datas):
            res[i][s.index] = d
        t2 = _time.perf_counter()
        self.stages = {"dispatch": t1 - t0, "fetch": t2 - t1}
        return {name: res[i].reshape(NCORES, *self.out_avals[i].shape)
                for i, name in enumerate(self.out_names)}


def kernel(x, seq_len, y, mask, emb, Wih_f, Whh_f, b_f, Wih_b, Whh_b, b_b,
           W_fc, start_t, end_t, trans):
    x = np.asarray(x); seq_len = np.asarray(seq_len); y = np.asarray(y)
    mask = np.asarray(mask)
    if "runner" not in _cache:
        nc = _build_nc()
        _cache["runner"] = _Runner(nc)
    r = _cache["runner"]

    _ts = _time.perf_counter()
    bld = _builders(x, seq_len, y, mask, emb, Wih_f, Whh_f, b_f, Wih_b,
                    Whh_b, b_b, W_fc, start_t, end_t, trans)
    for name, (key, fn) in bld.items():
        r.put(name, key, fn)
    _t0 = _time.perf_counter()
    res = r.run()
    kernel.last_device_s = _time.perf_counter() - _t0
    kernel.last_stages = {"stage_in": _t0 - _ts, **r.stages}
    kernel.last_results = res

    # ---- host: transition/start/end score part + loss assembly ----
    sc = res["out"]                    # [NCORES, 2, BL]
    logZ = sc[:, 0, :].reshape(B)
    emit = sc[:, 1, :].reshape(B)

    start_t = np.asarray(start_t, np.float32); end_t = np.asarray(end_t, np.float32)
    trans = np.asarray(trans, np.float32); yv = np.asarray(y).astype(np.int64)
    mf = mask.astype(np.float32)
    bidx = np.arange(B)
    trans_sc = trans[yv[:, :-1], yv[:, 1:]]
    last_tag = yv[bidx, np.asarray(seq_len).astype(np.int64) - 1]
    score = (start_t[yv[:, 0]] + (trans_sc * mf[:, 1:]).sum(1)
             + end_t[last_tag] + emit)
    return np.float32(-(score - logZ).sum())
